# revision 10
# baseline (speedup 1.0000x reference)
"""CrossAttention on 8 TRN2 NeuronCores (tensor-parallel over heads).

Reference computation (B=4, N=2048, DIM=1024, 16 heads, head_dim=64):
    qkv = x @ Wqkv.T + bqkv ; q, k = split(qkv)  (v unused)
    attn = softmax(q @ k.T * scale) ; out = attn @ split_heads(context)
    return merge_heads(out) @ Wout.T + bout

Sharding: core c owns heads {2c, 2c+1}. Each core computes q/k
projections for its heads (full sequence), head-parallel attention with
context slices as values, then a per-batch AllToAll re-shards from
head-parallel to row-parallel so the output projection runs locally.

Engine plan (per 512-query group): the PE emits the 16 score matmul
pairs (two heads on disjoint PE row groups run concurrently) finely
interleaved with the value matmuls of three-kc-older tiles plus
qkproj / outproj "filler" chunks, so the in-order PE stream never waits
long on softmax. Exp is split across engines: 11/16 tiles on ScalarE
(ActivationFunctionType.Exp) and 5/16 on VectorE via a bf16 Schraudolph
bit-trick (x*A+B -> int16 -> bitcast bf16; ~1.8% RMS per element, <1.1%
end-to-end after softmax). The softmax denominator comes from an
all-ones 65th value column; normalization is reciprocal_approx_fast +
DMA broadcast + one VectorE multiply reading PSUM directly.
"""
import numpy as np
import ml_dtypes

import concourse.bass as bass
import concourse.mybir as mybir
import concourse.tile as tile
from concourse import bacc
from concourse.bass_utils import run_bass_kernel_spmd

BF16 = ml_dtypes.bfloat16
F32 = mybir.dt.float32
BF = mybir.dt.bfloat16
I16 = mybir.dt.int16

NC = 8            # cores
B = 4             # batch
N = 2048          # sequence
DIM = 1024
NH = 16           # heads total
HD = 64           # head dim
HPC = NH // NC    # heads per core = 2
SCALE = HD ** -0.5
BN = B * N        # 8192 tokens
RPB = N // NC     # rows per (core, batch) after re-shard = 256
KC = DIM // 128   # contraction chunks for projections = 8
NKC = N // 128    # key chunks per batch = 16
CW = HD + 1       # value width incl. ones column = 65
NG = 4            # 512-query groups per batch
QTAG = 3          # live generations of q/k tiles

# bf16 Schraudolph exp: bitcast(int16(x*SCALE*184.665 + 16248.5))
SCH_A = 184.66502435 * SCALE
SCH_B = 16248.5


def build(DVE_KC=(3, 6, 9, 12, 15), LAG=4, FILL=True):
    nc = bacc.Bacc("TRN2", target_bir_lowering=False, debug=False,
                   num_devices=NC)

    xT = nc.dram_tensor("xT", [DIM, BN], BF, kind="ExternalInput")
    wqkT = nc.dram_tensor("wqkT", [DIM, 2 * 128], BF, kind="ExternalInput")
    bqk = nc.dram_tensor("bqk", [2 * 128, 1], F32, kind="ExternalInput")
    ctxa = nc.dram_tensor("ctxa", [B, HPC, 128, NKC * CW], BF,
                          kind="ExternalInput")
    woutT = nc.dram_tensor("woutT", [DIM, DIM], BF, kind="ExternalInput")
    boutb = nc.dram_tensor("boutb", [128, DIM], F32, kind="ExternalInput")
    out = nc.dram_tensor("out", [B * RPB, DIM], F32, kind="ExternalOutput")

    # per-batch AllToAll bounce buffers; chunk j holds rows
    # [j*256:(j+1)*256] of the covered batch
    a2a_in = [nc.dram_tensor(f"a2a_in{p}", [NC, 128, RPB], BF)
              for p in range(B)]
    a2a_out = [nc.dram_tensor(f"a2a_out{p}", [NC, 128, RPB], BF)
               for p in range(B)]
    rscr = [nc.dram_tensor(f"rscr{i}", [1, 1024], F32) for i in range(4)]

    with tile.TileContext(nc) as tc:
        with tc.tile_pool(name="const", bufs=1) as const, \
             tc.tile_pool(name="qk", bufs=1) as qkpool, \
             tc.tile_pool(name="xt", bufs=72) as xtpool, \
             tc.tile_pool(name="pt", bufs=18) as ptpool, \
             tc.tile_pool(name="r1", bufs=2) as r1pool, \
             tc.tile_pool(name="rb", bufs=3) as rbpool, \
             tc.tile_pool(name="ho", bufs=4) as hopool, \
             tc.tile_pool(name="sl", bufs=16) as slpool, \
             tc.tile_pool(name="ob", bufs=2) as obpool, \
             tc.tile_pool(name="pss", bufs=2, space="PSUM") as pss_pool, \
             tc.tile_pool(name="psm", bufs=4, space="PSUM") as psm_pool:

            wqk_sb = []
            for kc in range(KC):
                t = const.tile([128, 256], BF, tag=f"wqk{kc}")
                nc.sync.dma_start(out=t[:], in_=wqkT[kc * 128:(kc + 1) * 128, :])
                wqk_sb.append(t)
            bq_sb = []
            for fb in range(2):
                t = const.tile([128, 1], F32, tag=f"bq{fb}")
                nc.sync.dma_start(out=t[:], in_=bqk[fb * 128:(fb + 1) * 128, :])
                bq_sb.append(t)

            wout_sb = []
            bout_sb = const.tile([128, DIM], F32, tag="bout")
            ctx_sb = {}
            qk_tiles = {}
            xt_tiles = {}
            sl_tiles = {}
            _scr = [0]

            def load_out_consts():
                for fc in range(KC):
                    t = const.tile([128, DIM], BF, tag=f"wout{fc}",
                                   name=f"wout{fc}")
                    nc.sync.dma_start(
                        out=t[:], in_=woutT[fc * 128:(fc + 1) * 128, :])
                    wout_sb.append(t)
                nc.sync.dma_start(out=bout_sb[:], in_=boutb[:])

            def load_ctx(b):
                for h in range(HPC):
                    t = const.tile([128, NKC * CW], BF, tag=f"ctx{b}{h}",
                                   name=f"ctx{b}_{h}")
                    nc.sync.dma_start(out=t[:], in_=ctxa[b, h, :, :])
                    ctx_sb[b, h] = t

            def prefetch_x(b):
                qT = qkpool.tile([128, N], BF, tag=f"qT{b % QTAG}",
                                 name=f"qT{b}")
                kT = qkpool.tile([128, N], BF, tag=f"kT{b % QTAG}",
                                 name=f"kT{b}")
                qk_tiles[b] = (qT, kT)
                xts = {}
                for t in range(4):
                    for kc in range(KC):
                        xt = xtpool.tile([128, 512], BF, tag="xt",
                                         name=f"xtb{b}_{kc}_{t}")
                        eng = nc.sync if kc % 2 else nc.gpsimd
                        eng.dma_start(
                            out=xt[:], in_=xT[kc * 128:(kc + 1) * 128,
                                              b * N + t * 512:
                                              b * N + (t + 1) * 512])
                        xts[kc, t] = xt
                xt_tiles[b] = xts

            def qk_half(b, t, fb):
                """Project one 512-token chunk for q (fb=0) or k (fb=1)."""
                qT, kT = qk_tiles[b]
                dst = kT if fb else qT
                ps = psm_pool.tile([128, 512], F32, tag="psm",
                                   name=f"psq{b}_{t}_{fb}")
                for kc in range(KC):
                    nc.tensor.matmul(
                        ps[:], wqk_sb[kc][:, fb * 128:(fb + 1) * 128],
                        xt_tiles[b][kc, t][:],
                        start=(kc == 0), stop=(kc == KC - 1))
                nc.vector.tensor_scalar_add(
                    dst[:, t * 512:(t + 1) * 512], ps[:], bq_sb[fb][:])

            def out_sl(b, rc):
                """Prefetch the a2a_out slices for one outproj row chunk."""
                sls = []
                for fc in range(KC):
                    sl = slpool.tile([128, 128], BF, tag="sl",
                                     name=f"sl{b}_{rc}_{fc}")
                    eng = nc.sync if fc % 2 else nc.gpsimd
                    eng.dma_start(
                        out=sl[:],
                        in_=a2a_out[b][fc, :, rc * 128:(rc + 1) * 128])
                    sls.append(sl)
                sl_tiles[b, rc] = sls

            def out_rc(b, rc):
                """Output projection for one 128-row chunk of batch b."""
                sls = sl_tiles.pop((b, rc))
                pso = [psm_pool.tile([128, 512], F32, tag="psm",
                                     name=f"pso{b}_{rc}_{i}")
                       for i in range(2)]
                for fc in range(KC):
                    for n in range(2):
                        nc.tensor.matmul(
                            pso[n][:], sls[fc][:],
                            wout_sb[fc][:, n * 512:(n + 1) * 512],
                            start=(fc == 0), stop=(fc == KC - 1))
                for n in range(2):
                    ob = obpool.tile([128, 512], F32, tag="ob",
                                     name=f"ob{b}_{rc}_{n}")
                    nc.vector.tensor_tensor(
                        out=ob[:], in0=pso[n][:],
                        in1=bout_sb[:, n * 512:(n + 1) * 512],
                        op=mybir.AluOpType.add)
                    nc.sync.dma_start(
                        out=out[b * RPB + rc * 128:b * RPB + (rc + 1) * 128,
                                n * 512:(n + 1) * 512],
                        in_=ob[:])

            # ---- filler queue: (kind, cost, fn) PE chunks that hide ----
            # ---- softmax latency inside attention groups            ----
            fillers = []

            def pop_fillers(budget):
                while fillers and fillers[0][1] <= budget:
                    kind, cost, fn = fillers.pop(0)
                    fn()
                    budget -= cost
                return budget

            def drain_qk():
                """Emit remaining qk units (their batch starts next)."""
                while fillers and fillers[0][0] == "qk":
                    fillers.pop(0)[2]()

            def attn_group(b, g, startup=False):
                """Scores+softmax+values for 512 queries, both heads."""
                qT, kT = qk_tiles[b]
                q0 = g * 512
                pts = [None] * NKC
                pavs = None
                budget = [0 if startup else 3]
                av_lag = NKC if startup else LAG

                def av(kc):
                    for h in range(HPC):
                        nc.tensor.matmul(
                            pavs[h][:], ctx_sb[b, h][:, kc * CW:(kc + 1) * CW],
                            pts[kc][:, h * 512:(h + 1) * 512],
                            start=(kc == 0), stop=(kc == NKC - 1))

                for kc in range(NKC):
                    if startup and kc % 4 == 0 and kc > 0:
                        t = kc // 4
                        qk_half(b, t, 1)
                        qk_half(b, t, 0)
                    ps = pss_pool.tile([128, 1024], F32, tag="pss",
                                       name=f"pss{b}{g}{kc}")
                    for h in range(HPC):
                        nc.tensor.matmul(
                            ps[:, h * 512:(h + 1) * 512],
                            kT[h * HD:(h + 1) * HD, kc * 128:(kc + 1) * 128],
                            qT[h * HD:(h + 1) * HD, q0:q0 + 512],
                            start=True, stop=True,
                            tile_position=(h * HD, 0))
                    pt = ptpool.tile([128, 1024], BF, tag="pt",
                                     name=f"pt{b}{g}{kc}")
                    pts[kc] = pt
                    if kc in DVE_KC:
                        nc.vector.tensor_scalar(
                            pt[:].bitcast(I16), ps[:], SCH_A, SCH_B,
                            op0=mybir.AluOpType.mult,
                            op1=mybir.AluOpType.add)
                    else:
                        nc.scalar.activation(
                            pt[:], ps[:],
                            mybir.ActivationFunctionType.Exp, scale=SCALE)
                    if kc == av_lag:
                        pavs = [psm_pool.tile([CW, 512], F32, tag="psm",
                                              name=f"pav{b}{g}{h}")
                                for h in range(HPC)]
                    if FILL and kc in (3, 8, 13):
                        budget[0] = pop_fillers(budget[0])
                    if kc >= av_lag:
                        av(kc - av_lag)
                if pavs is None:
                    pavs = [psm_pool.tile([CW, 512], F32, tag="psm",
                                          name=f"pav{b}{g}{h}")
                            for h in range(HPC)]
                for kc in range(NKC - av_lag, NKC):
                    av(kc)
                # normalize both heads: 1/colsum -> broadcast -> multiply
                scr = rscr[_scr[0] % 4]
                _scr[0] += 1
                for h in range(HPC):
                    sden = r1pool.tile([1, 512], F32, tag="sden",
                                       name=f"sden{b}{g}{h}")
                    # reciprocal_approx_fast mis-reads partition-base-64
                    # APs; stage the denominator row at base 0 first
                    nc.vector.tensor_copy(sden[:], pavs[h][HD:CW, :])
                    r1 = r1pool.tile([1, 512], F32, tag="r1",
                                     name=f"r1{b}{g}{h}")
                    nc.vector.reciprocal_approx_fast(r1[:], sden[:])
                    nc.gpsimd.dma_start(out=scr[:, h * 512:(h + 1) * 512],
                                        in_=r1[:])
                rb = rbpool.tile([HD, 1024], F32, tag="rb", name=f"rb{b}{g}")
                nc.gpsimd.dma_start(out=rb[:],
                                    in_=scr[:].broadcast_to([HD, 1024]))
                for h in range(HPC):
                    ho = hopool.tile([HD, 512], BF, tag="ho",
                                     name=f"ho{b}{g}{h}")
                    nc.vector.tensor_tensor(
                        out=ho[:], in0=pavs[h][0:HD, :],
                        in1=rb[:, h * 512:(h + 1) * 512],
                        op=mybir.AluOpType.mult)
                    for half in range(2):
                        j = (q0 + half * 256) // RPB
                        nc.gpsimd.dma_start(
                            out=a2a_in[b][j, h * HD:(h + 1) * HD, :],
                            in_=ho[:, half * 256:(half + 1) * 256])

            def reshard(p):
                nc.gpsimd.collective_compute(
                    "AllToAll", mybir.AluOpType.bypass,
                    replica_groups=[list(range(NC))],
                    ins=[a2a_in[p].ap().opt()], outs=[a2a_out[p].ap().opt()])

            # ---- schedule ----
            prefetch_x(0)
            load_ctx(0)
            prefetch_x(1)
            load_ctx(1)
            qk_half(0, 0, 1)
            qk_half(0, 0, 0)
            load_out_consts()

            for b in range(B):
                if b + 2 < B:
                    prefetch_x(b + 2)
                    load_ctx(b + 2)
                # queue qkproj of batch b+1 (must fully emit this batch),
                # then outproj of batch b-1 (resharded already, can spill)
                if b + 1 < B:
                    for t in range(4):
                        for fb in (1, 0):
                            fillers.append(
                                ("qk", 1, lambda b_=b + 1, t_=t, f_=fb:
                                 qk_half(b_, t_, f_)))
                if b >= 1:
                    fillers.append(("sl", 1, lambda bb=b - 1: out_sl(bb, 0)))
                    fillers.append(("out", 2, lambda bb=b - 1: out_rc(bb, 0)))
                    if b < B - 1:
                        fillers.append(
                            ("sl", 1, lambda bb=b - 1: out_sl(bb, 1)))
                        fillers.append(
                            ("out", 2, lambda bb=b - 1: out_rc(bb, 1)))
                for g in range(NG):
                    attn_group(b, g, startup=(b == 0 and g == 0))
                    if not FILL:
                        pop_fillers(3)
                drain_qk()
                reshard(b)
            out_sl(B - 2, 1)
            for kind, cost, fn in fillers:
                fn()
            out_sl(B - 1, 0)
            out_sl(B - 1, 1)
            out_rc(B - 2, 1)
            out_rc(B - 1, 0)
            out_rc(B - 1, 1)
    nc.compile()
    return nc


def prep_inputs(x, context, Wqkv, bqkv, Wout, bout):
    """Host-side sharding: returns in_maps for the 8 cores."""
    x = np.asarray(x, np.float32)
    context = np.asarray(context, np.float32)
    Wqkv = np.asarray(Wqkv, np.float32)
    bqkv = np.asarray(bqkv, np.float32)
    Wout = np.asarray(Wout, np.float32)
    bout = np.asarray(bout, np.float32)

    xT = np.ascontiguousarray(x.reshape(BN, DIM).T).astype(BF16)
    woutT = np.ascontiguousarray(Wout.T).astype(BF16)
    boutb = np.broadcast_to(bout, (128, DIM)).astype(np.float32).copy()

    in_maps = []
    for c in range(NC):
        h0 = c * HPC
        wq = Wqkv[h0 * HD:(h0 + HPC) * HD]
        wk = Wqkv[DIM + h0 * HD:DIM + (h0 + HPC) * HD]
        wqkT = np.ascontiguousarray(
            np.concatenate([wq, wk], axis=0).T).astype(BF16)
        bq = np.concatenate([bqkv[h0 * HD:(h0 + HPC) * HD],
                             bqkv[DIM + h0 * HD:DIM + (h0 + HPC) * HD]])
        bq = bq.reshape(2 * 128, 1).astype(np.float32)
        ctxa = np.ones((B, HPC, 128, NKC, CW), np.float32)
        for h in range(HPC):
            g = h0 + h
            arr = context[:, :, g * HD:(g + 1) * HD].reshape(B, NKC, 128, HD)
            ctxa[:, h, :, :, :HD] = arr.transpose(0, 2, 1, 3)
        in_maps.append({
            "xT": xT,
            "wqkT": wqkT,
            "bqk": bq,
            "ctxa": ctxa.reshape(B, HPC, 128, NKC * CW).astype(BF16),
            "woutT": woutT,
            "boutb": boutb,
        })
    return in_maps


_NC_CACHE = None


def _get_nc():
    global _NC_CACHE
    if _NC_CACHE is None:
        _NC_CACHE = build()
    return _NC_CACHE


def run(in_maps, trace=False):
    nc = _get_nc()
    res = run_bass_kernel_spmd(nc, in_maps, core_ids=list(range(NC)),
                               trace=trace)
    full = np.empty((B, N, DIM), np.float32)
    for c in range(NC):
        o = np.asarray(res.results[c]["out"]).reshape(B, RPB, DIM)
        full[:, c * RPB:(c + 1) * RPB, :] = o
    return full, res


def kernel(x, context, Wqkv, bqkv, Wout, bout):
    in_maps = prep_inputs(x, context, Wqkv, bqkv, Wout, bout)
    out, _ = run(in_maps, trace=False)
    return out
